# revision 12
# baseline (speedup 1.0000x reference)
"""Trainium2 Bass kernel for nn_MiniResNetJoint (topk_masking).

Data-parallel over 8 NeuronCores (16 images each). BN batch statistics are
synchronized across cores with small AllGather collectives (7 rounds).
Conv trunk + 5 linear heads run on device with fp32r (s1e8m11) matmuls and
fp32 accumulation; the tiny symbolic reranker tail runs on host numpy.
"""

import numpy as np

import concourse.bass as bass
import concourse.mybir as mybir
import concourse.tile as tile
from concourse import bacc
from concourse.bass_utils import run_bass_kernel_spmd

F32 = mybir.dt.float32
F32R = mybir.dt.float32r
AX = mybir.AxisListType
OP = mybir.AluOpType
ACTF = mybir.ActivationFunctionType

N_CORES = 8
BL = 16            # local images per core
HALF = 8           # images per partition-half for 64ch layers
NUM_CLASSES = 3755
NUM_RADICALS = 214
NUM_STRUCT = 13
NUM_SC = 30
NUM_ST = 6
TOP_K = 5
EPS = 1e-5

_CACHE = {}


def _round_f32r(x):
    xi = np.ascontiguousarray(x, dtype=np.float32).view(np.uint32).astype(np.uint64)
    shift = np.uint64(12)
    add = np.uint64((1 << 11) - 1)
    lsb = (xi >> shift) & np.uint64(1)
    xi = (((xi + add + lsb) >> shift) << shift).astype(np.uint32)
    return xi.view(np.float32).reshape(np.shape(x))


# ---------------------------------------------------------------- host prep

def _prep_consts(params):
    p = {k: np.asarray(v) for k, v in params.items()}
    c = {}

    ws = np.asarray(p["stem_w"], np.float32)[:, 0]          # (64,3,3)
    lst = np.zeros((18, 128), np.float32)
    for t in range(9):
        dh, dw = divmod(t, 3)
        lst[t, :64] = ws[:, dh, dw]
        lst[9 + t, 64:] = ws[:, dh, dw]
    c["WSTEM"] = _round_f32r(lst)

    def blockdiag(w_tap):                                   # (64,64)->(128,128)
        out = np.zeros((128, 128), np.float32)
        out[:64, :64] = w_tap.T
        out[64:, 64:] = w_tap.T
        return out

    def duprows(w_tap):                                     # (128,64)->(128,128)
        out = np.zeros((128, 128), np.float32)
        out[:64, :] = w_tap.T
        out[64:, :] = w_tap.T
        return out

    for name, key in (("W1C1", "b1_c1"), ("W1C2", "b1_c2")):
        w = np.asarray(p[key], np.float32)
        taps = np.stack([blockdiag(w[:, :, t // 3, t % 3]) for t in range(9)])
        c[name] = _round_f32r(taps.transpose(1, 0, 2).reshape(128, 9 * 128))

    w = np.asarray(p["b2_c1"], np.float32)
    taps = np.stack([duprows(w[:, :, t // 3, t % 3]) for t in range(9)])
    c["W2C1"] = _round_f32r(taps.transpose(1, 0, 2).reshape(128, 9 * 128))
    c["W2SKIP"] = _round_f32r(duprows(np.asarray(p["b2_skip_w"], np.float32)[:, :, 0, 0]))

    w = np.asarray(p["b2_c2"], np.float32)
    taps = np.stack([w[:, :, t // 3, t % 3].T for t in range(9)])
    c["W2C2"] = _round_f32r(taps.transpose(1, 0, 2).reshape(128, 9 * 128))

    w = np.asarray(p["b3_c1"], np.float32)
    arr = np.zeros((9, 2, 128, 128), np.float32)
    for t in range(9):
        for mh in range(2):
            arr[t, mh] = w[mh * 128:(mh + 1) * 128, :, t // 3, t % 3].T
    c["W3C1"] = _round_f32r(arr.reshape(18, 128, 128).transpose(1, 0, 2)
                            .reshape(128, 18 * 128))

    w = np.asarray(p["b3_skip_w"], np.float32)[:, :, 0, 0]
    arr = np.stack([w[:128].T, w[128:].T])
    c["W3SKIP"] = _round_f32r(arr.transpose(1, 0, 2).reshape(128, 2 * 128))

    w = np.asarray(p["b3_c2"], np.float32)
    arr = np.zeros((9, 2, 2, 128, 128), np.float32)
    for t in range(9):
        for kc in range(2):
            for mh in range(2):
                arr[t, kc, mh] = w[mh * 128:(mh + 1) * 128,
                                   kc * 128:(kc + 1) * 128, t // 3, t % 3].T
    c["W3C2"] = _round_f32r(arr.reshape(36, 128, 128).transpose(1, 0, 2)
                            .reshape(128, 36 * 128))

    def head(wk_, mpad):
        w_ = np.asarray(p[wk_], np.float32) / 64.0          # GAP fold
        o = w_.shape[0]
        arr_ = np.zeros((2, 128, mpad), np.float32)
        for kc in range(2):
            arr_[kc, :, :o] = w_[:, kc * 128:(kc + 1) * 128].T
        return _round_f32r(arr_.transpose(1, 0, 2).reshape(128, 2 * mpad))

    c["WCHAR"] = head("char_w", 3840)
    c["WSMALL"] = np.concatenate([head("rad_w", 256), head("sc_w", 32),
                                  head("st_w", 8), head("str_w", 16)], axis=1)

    bc = np.zeros((30, 128), np.float32)
    bc.reshape(-1)[:NUM_CLASSES] = np.asarray(p["char_b"], np.float32)
    c["BCHAR"] = np.ascontiguousarray(bc.T)                  # [128, 30]
    bsm = np.zeros((128, 4), np.float32)
    rb = np.asarray(p["rad_b"], np.float32)
    bsm[:128, 0] = rb[:128]
    bsm[:86, 1] = rb[128:]
    bsm[:30, 2] = np.asarray(p["sc_b"], np.float32)
    bsm[:6, 3] = np.asarray(p["st_b"], np.float32)
    c["BSM"] = bsm
    bstr = np.zeros((128, 1), np.float32)
    bstr[:13, 0] = np.asarray(p["str_b"], np.float32)
    c["BSTR"] = bstr

    def gb_dup(g, b_):
        g = np.asarray(g, np.float32); b_ = np.asarray(b_, np.float32)
        return np.stack([np.concatenate([g, g]), np.concatenate([b_, b_])], 1)

    def gb_full(g, b_):
        return np.stack([np.asarray(g, np.float32), np.asarray(b_, np.float32)], 1)

    c["ZPAD"] = np.zeros((128, 34 * 34 * 8), np.float32)
    c["GBALL"] = np.concatenate([
        gb_dup(p["stem_g"], p["stem_b"]),
        gb_dup(p["b1_g1"], p["b1_beta1"]),
        gb_dup(p["b1_g2"], p["b1_beta2"]),
        gb_full(p["b2_g1"], p["b2_beta1"]),
        gb_full(p["b2_skip_g"], p["b2_skip_b"]),
        gb_full(p["b2_g2"], p["b2_beta2"]),
        gb_full(p["b3_g1"][:128], p["b3_beta1"][:128]),
        gb_full(p["b3_g1"][128:], p["b3_beta1"][128:]),
        gb_full(p["b3_skip_g"][:128], p["b3_skip_b"][:128]),
        gb_full(p["b3_skip_g"][128:], p["b3_skip_b"][128:]),
        gb_full(p["b3_g2"][:128], p["b3_beta2"][:128]),
        gb_full(p["b3_g2"][128:], p["b3_beta2"][128:]),
    ], axis=1).astype(np.float32)                           # [128, 24]
    return c


def _prep_stem_rhs(x_core):
    """(16,1,64,64) -> [8(pass),18,4356] f32r rolled zero-padded planes."""
    x = np.asarray(x_core, np.float32)[:, 0]
    xp = np.zeros((16, 66, 66), np.float32)
    xp[:, 1:65, 1:65] = x
    flat = xp.reshape(16, 66 * 66)
    out = np.zeros((8, 18, 4356), np.float32)
    for ps in range(8):
        for half in range(2):
            img = half * 8 + ps
            for t in range(9):
                dh, dw = divmod(t, 3)
                s = dh * 66 + dw
                seg = flat[img, s:s + 4356]
                out[ps, half * 9 + t, :len(seg)] = seg
    return _round_f32r(out)


# ---------------------------------------------------------------- device build

def _build(n_cores):
    nc = bacc.Bacc("TRN2", target_bir_lowering=False, debug=False,
                   num_devices=n_cores)

    def din(name, shape, dtype=F32R):
        return nc.dram_tensor(name, list(shape), dtype, kind="ExternalInput").ap()

    X0 = din("X0", (8, 18, 4356))
    WSTEM = din("WSTEM", (18, 128))
    W1C1 = din("W1C1", (128, 9 * 128))
    W1C2 = din("W1C2", (128, 9 * 128))
    W2C1 = din("W2C1", (128, 9 * 128))
    W2SKIP = din("W2SKIP", (128, 128))
    W2C2 = din("W2C2", (128, 9 * 128))
    W3C1 = din("W3C1", (128, 18 * 128))
    W3SKIP = din("W3SKIP", (128, 2 * 128))
    W3C2 = din("W3C2", (128, 36 * 128))
    WCHAR = din("WCHAR", (128, 2 * 3840))
    WSMALL = din("WSMALL", (128, 2 * 312))
    BCHAR = din("BCHAR", (128, 30), F32)
    BSM = din("BSM", (128, 4), F32)
    BSTR = din("BSTR", (128, 1), F32)
    GBALL = din("GBALL", (128, 24), F32)
    ZPAD = din("ZPAD", (128, 34 * 34 * 8))

    CHAR = nc.dram_tensor("CHAR", [BL, NUM_CLASSES], F32, kind="ExternalOutput").ap()
    RAD = nc.dram_tensor("RAD", [BL, NUM_RADICALS], F32, kind="ExternalOutput").ap()
    SCO = nc.dram_tensor("SCO", [BL, NUM_SC], F32, kind="ExternalOutput").ap()
    STO = nc.dram_tensor("STO", [BL, NUM_ST], F32, kind="ExternalOutput").ap()
    STRO = nc.dram_tensor("STRO", [BL, NUM_STRUCT], F32, kind="ExternalOutput").ap()

    rg = [list(range(n_cores))]

    with tile.TileContext(nc) as tc:
        with (
            tc.tile_pool(name="mem", bufs=1) as mem,
            tc.tile_pool(name="psum", bufs=6, space="PSUM") as pp,
            tc.tile_pool(name="psum2", bufs=2, space="PSUM") as pps,
            tc.tile_pool(name="dram", bufs=2, space="DRAM") as dpool,
            tc.tile_pool(name="hout", bufs=4) as hpool,
            tc.tile_pool(name="stats", bufs=1) as spool,
        ):
            tctr = [0]

            def mtile(shape, slot, dtype=F32):
                tctr[0] += 1
                return mem.tile(list(shape), dtype, tag=slot,
                                name=f"{slot}_{tctr[0]}")

            def load(ap_dram, shape, slot, dtype=F32R):
                t = mtile(shape, slot, dtype)
                nc.sync.dma_start(t[:], ap_dram)
                return t

            # persistent small constants
            gball = load(GBALL, (128, 24), "gball", F32)

            uid = [0]

            def stile(shape, name):
                return spool.tile(list(shape), F32, tag=name, name=name)

            def rsqrt_newton(out_ap, var_ap):
                uid[0] += 1
                u = uid[0]
                t = stile((128, 1), f"rsq_t{u}")
                s = stile((128, 1), f"rsq_s{u}")
                r0 = stile((128, 1), f"rsq_r0{u}")
                h = stile((128, 1), f"rsq_h{u}")
                nc.vector.tensor_scalar_add(t[:], var_ap, float(EPS))
                nc.scalar.activation(s[:], t[:], ACTF.Sqrt)
                nc.vector.reciprocal(r0[:], s[:])
                nc.vector.tensor_tensor(h[:], r0[:], r0[:], op=OP.mult)
                nc.vector.tensor_tensor(h[:], h[:], t[:], op=OP.mult)
                nc.vector.tensor_scalar(h[:], h[:], -0.5, 1.5, op0=OP.mult,
                                        op1=OP.add)
                nc.vector.tensor_tensor(out_ap, r0[:], h[:], op=OP.mult)

            def bn_sync(stat_groups, gb_cols, dup_halves):
                """Returns [(a_tile, b_tile)] per group; a/b are [128,1] fp32."""
                uid[0] += 1
                u = uid[0]
                L = len(stat_groups)
                mv = stile((128, 2 * L), f"mv{u}")
                for gi, sg in enumerate(stat_groups):
                    ag = stile((128, 2), f"aggr{u}_{gi}")
                    nc.vector.bn_aggr(ag[:], sg)
                    nc.vector.tensor_copy(mv[:, 2 * gi:2 * gi + 1], ag[:, 0:1])
                    m2 = stile((128, 1), f"m2_{u}_{gi}")
                    nc.vector.tensor_tensor(m2[:], ag[:, 0:1], ag[:, 0:1],
                                            op=OP.mult)
                    nc.vector.tensor_tensor(mv[:, 2 * gi + 1:2 * gi + 2],
                                            ag[:, 1:2], m2[:], op=OP.add)
                agin = dpool.tile([128, 2 * L], F32, tag="agin", name=f"agin{u}")
                agout = dpool.tile([n_cores * 128, 2 * L], F32,
                                   addr_space="Shared", tag="agout",
                                   name=f"agout{u}")
                nc.sync.dma_start(agin[:], mv[:])
                if n_cores > 1:
                    nc.gpsimd.collective_compute(
                        "AllGather", OP.bypass, replica_groups=rg,
                        ins=[agin[:].opt()], outs=[agout[:].opt()])
                else:
                    nc.sync.dma_start(agout[:], agin[:])
                gt = stile((128, 2 * L * n_cores), f"gt{u}")
                nc.sync.dma_start(
                    gt[:], agout[:].rearrange("(r p) j -> p j r", r=n_cores))
                red = stile((128, 2 * L), f"red{u}")
                nc.vector.tensor_reduce(
                    red[:], gt[:].rearrange("p (j r) -> p j r", r=n_cores),
                    axis=AX.X, op=OP.add)
                if dup_halves:
                    up = stile((64, 2 * L), f"up{u}")
                    nc.sync.dma_start(up[:], red[64:128, :])
                    nc.vector.tensor_tensor(red[0:64, :], red[0:64, :], up[:],
                                            op=OP.add)
                    nc.vector.tensor_scalar_mul(red[0:64, :], red[0:64, :],
                                                1.0 / (2 * n_cores))
                    nc.sync.dma_start(red[64:128, :], red[0:64, :])
                else:
                    nc.vector.tensor_scalar_mul(red[:], red[:], 1.0 / n_cores)
                out = []
                for gi in range(L):
                    m = red[:, 2 * gi:2 * gi + 1]
                    ex2 = red[:, 2 * gi + 1:2 * gi + 2]
                    var = stile((128, 1), f"var{u}_{gi}")
                    nc.vector.tensor_tensor(var[:], m, m, op=OP.mult)
                    nc.vector.tensor_tensor(var[:], ex2, var[:], op=OP.subtract)
                    r = stile((128, 1), f"r{u}_{gi}")
                    rsqrt_newton(r[:], var[:])
                    a = stile((128, 1), f"a{u}_{gi}")
                    b = stile((128, 1), f"b{u}_{gi}")
                    gcol = gb_cols[gi]
                    nc.vector.tensor_tensor(a[:], gball[:, 2 * gcol:2 * gcol + 1],
                                            r[:], op=OP.mult)
                    nc.vector.tensor_tensor(b[:], m, a[:], op=OP.mult)
                    nc.vector.tensor_tensor(b[:],
                                            gball[:, 2 * gcol + 1:2 * gcol + 2],
                                            b[:], op=OP.subtract)
                    out.append((a, b))
                return out

            def pool2x2_all(src4, dst4, n_img, hw, slot):
                h2 = hw // 2
                tmp = mtile((128, n_img * hw * h2), slot)
                tv = tmp[:].rearrange("p (i h w) -> p i h w", i=n_img, h=hw)
                s = src4.rearrange("p i h (w2 two) -> p i h w2 two", two=2)
                nc.vector.tensor_tensor(tv, s[:, :, :, :, 0], s[:, :, :, :, 1],
                                        op=OP.max)
                t2 = tv.rearrange("p i (h2 two) w -> p i h2 two w", two=2)
                nc.vector.tensor_tensor(dst4, t2[:, :, :, 0, :],
                                        t2[:, :, :, 1, :], op=OP.max)

            # ================= STEM =================
            wstem = load(WSTEM, (18, 128), "wstem")
            b1pad = mtile((128, HALF * 34 * 34), "padA", F32R)
            nc.sync.dma_start(b1pad[:], ZPAD[:, 0:HALF * 34 * 34])
            b1padv = b1pad[:].rearrange("p (i h w) -> p i h w", i=HALF, h=34)
            st_stats = stile((128, 64 * 6), "st_stats")

            for ps in range(8):
                x0t = load(X0[ps], (18, 4356), "sA")
                raw = mtile((128, 4096), "sB")
                rawv = raw[:].rearrange("p (h w) -> p h w", h=64)
                for hb in range(8):
                    psm = pp.tile([128, 512], F32, tag="psm", name=f"ps_st{ps}_{hb}")
                    rhs = x0t[:, hb * 528:(hb + 1) * 528].rearrange(
                        "p (h w) -> p h w", h=8)[:, :, 0:64]
                    nc.tensor.matmul(psm[:], wstem[:], rhs, start=True, stop=True)
                    ci = ps * 8 + hb
                    nc.vector.bn_stats(st_stats[:, ci * 6:(ci + 1) * 6], psm[:])
                    nc.scalar.copy(rawv[:, hb * 8:(hb + 1) * 8, :],
                                   psm[:].rearrange("p (h w) -> p h w", h=8))
                dst = b1padv[:, ps:ps + 1, 1:33, 1:33]
                pool2x2_all(rawv.rearrange("p h w -> p 1 h w" if False else
                                           "p h w -> p h w").unsqueeze(1)
                            if False else
                            raw[:].rearrange("p (i h w) -> p i h w", i=1, h=64),
                            dst, 1, 64, "sC")

            [(a0, b0)] = bn_sync([st_stats[:]], [0], True)
            intr = b1padv[:, :, 1:33, 1:33]
            nc.scalar.activation(intr, intr.bitcast(F32), ACTF.Relu,
                                 bias=b0[:], scale=a0[:])

            # ================= B1 =================
            def conv64(wtile, srcv, n_img, hw, dstv, stats_tile):
                rpc = 512 // hw
                nch = hw // rpc
                ci = 0
                for i in range(n_img):
                    for cch in range(nch):
                        psm = pp.tile([128, rpc * hw], F32, tag="psm",
                                      name=f"psc64_{tctr[0]}_{ci}")
                        for t in range(9):
                            dh, dw = divmod(t, 3)
                            rhs = srcv[:, i, dh + cch * rpc: dh + cch * rpc + rpc,
                                       dw:dw + hw]
                            nc.tensor.matmul(psm[:],
                                             wtile[:, t * 128:(t + 1) * 128],
                                             rhs, start=(t == 0), stop=(t == 8))
                        nc.vector.bn_stats(stats_tile[:, ci * 6:(ci + 1) * 6],
                                           psm[:])
                        nc.scalar.copy(
                            dstv[:, i, cch * rpc:(cch + 1) * rpc, :],
                            psm[:].rearrange("p (h w) -> p h w", h=rpc))
                        ci += 1

            w1c1 = load(W1C1, (128, 9 * 128), "wconv")
            s_b1c1 = stile((128, 16 * 6), "s_b1c1")
            y1 = mtile((128, HALF * 1024), "sA")
            y1v = y1[:].rearrange("p (i h w) -> p i h w", i=HALF, h=32)
            conv64(w1c1, b1padv, HALF, 32, y1v, s_b1c1)
            [(a1, b1)] = bn_sync([s_b1c1[:]], [1], True)
            y1pad = mtile((128, HALF * 34 * 34), "padB", F32R)
            nc.sync.dma_start(y1pad[:], ZPAD[:, 0:HALF * 34 * 34])
            y1padv = y1pad[:].rearrange("p (i h w) -> p i h w", i=HALF, h=34)
            nc.scalar.activation(y1padv[:, :, 1:33, 1:33], y1v, ACTF.Relu,
                                 bias=b1[:], scale=a1[:])

            w1c2 = load(W1C2, (128, 9 * 128), "wconv")
            s_b1c2 = stile((128, 16 * 6), "s_b1c2")
            y2 = mtile((128, HALF * 1024), "sB")
            y2v = y2[:].rearrange("p (i h w) -> p i h w", i=HALF, h=32)
            conv64(w1c2, y1padv, HALF, 32, y2v, s_b1c2)
            [(a2, b2)] = bn_sync([s_b1c2[:]], [2], True)
            zb1 = mtile((128, HALF * 1024), "sA")
            zb1v = zb1[:].rearrange("p (i h w) -> p i h w", i=HALF, h=32)
            nc.vector.tensor_scalar_add(zb1v, intr.bitcast(F32), b2[:])
            nc.vector.scalar_tensor_tensor(zb1[:], y2[:], a2[:], zb1[:],
                                           op0=OP.mult, op1=OP.add)
            b2pad = mtile((128, HALF * 18 * 18), "padA", F32R)
            nc.sync.dma_start(b2pad[:], ZPAD[:, 0:HALF * 18 * 18])
            b2padv = b2pad[:].rearrange("p (i h w) -> p i h w", i=HALF, h=18)
            zp1 = mtile((128, HALF * 256), "sE")
            zp1v = zp1[:].rearrange("p (i h w) -> p i h w", i=HALF, h=16)
            pool2x2_all(zb1v, zp1v, HALF, 32, "sC")
            nc.scalar.activation(b2padv[:, :, 1:17, 1:17], zp1v, ACTF.Relu)

            # ================= B2 =================
            w2c1 = load(W2C1, (128, 9 * 128), "wconv")
            w2skip = load(W2SKIP, (128, 128), "wskip")
            s_b2c1 = stile((128, 8 * 6), "s_b2c1")
            s_b2sk = stile((128, 8 * 6), "s_b2sk")
            y21 = mtile((128, BL * 256), "sA")
            sk2 = mtile((128, BL * 256), "sB")
            y21v = y21[:].rearrange("p (i h w) -> p i h w", i=BL, h=16)
            sk2v = sk2[:].rearrange("p (i h w) -> p i h w", i=BL, h=16)
            ci1 = 0
            for half in range(2):
                hp = half * 64
                for cc in range(4):
                    i0 = cc * 2
                    img = half * 8 + i0
                    psm = pp.tile([128, 512], F32, tag="psm", name=f"psb2c1_{ci1}")
                    for t in range(9):
                        dh, dw = divmod(t, 3)
                        rhs = b2padv[hp:hp + 64, i0:i0 + 2, dh:dh + 16,
                                     dw:dw + 16]
                        nc.tensor.matmul(psm[:],
                                         w2c1[hp:hp + 64, t * 128:(t + 1) * 128],
                                         rhs, start=(t == 0), stop=(t == 8))
                    nc.vector.bn_stats(s_b2c1[:, ci1 * 6:(ci1 + 1) * 6], psm[:])
                    nc.scalar.copy(y21v[:, img:img + 2, :, :],
                                   psm[:].rearrange("p (i h w) -> p i h w",
                                                    i=2, h=16))
                    psk = pps.tile([128, 512], F32, tag="psk", name=f"psb2sk_{ci1}")
                    rhs = b2padv[hp:hp + 64, i0:i0 + 2, 1:17, 1:17]
                    nc.tensor.matmul(psk[:], w2skip[hp:hp + 64, :], rhs,
                                     start=True, stop=True)
                    nc.vector.bn_stats(s_b2sk[:, ci1 * 6:(ci1 + 1) * 6], psk[:])
                    nc.scalar.copy(sk2v[:, img:img + 2, :, :],
                                   psk[:].rearrange("p (i h w) -> p i h w",
                                                    i=2, h=16))
                    ci1 += 1
            ab21, absk = bn_sync([s_b2c1[:], s_b2sk[:]], [3, 4], False)
            b2c1pad = mtile((128, BL * 18 * 18), "padB", F32R)
            nc.sync.dma_start(b2c1pad[:], ZPAD[:, 0:BL * 18 * 18])
            cv = b2c1pad[:].rearrange("p (i h w) -> p i h w", i=BL, h=18)
            nc.scalar.activation(cv[:, :, 1:17, 1:17], y21v, ACTF.Relu,
                                 bias=ab21[1][:], scale=ab21[0][:])
            nc.vector.tensor_scalar(sk2[:], sk2[:], absk[0][:], absk[1][:],
                                    op0=OP.mult, op1=OP.add)

            w2c2 = load(W2C2, (128, 9 * 128), "wconv")
            s_b2c2 = stile((128, 8 * 6), "s_b2c2")
            y22 = mtile((128, BL * 256), "sC")
            y22v = y22[:].rearrange("p (i h w) -> p i h w", i=BL, h=16)
            for cc in range(8):
                i0 = cc * 2
                psm = pp.tile([128, 512], F32, tag="psm", name=f"psb2c2_{cc}")
                for t in range(9):
                    dh, dw = divmod(t, 3)
                    rhs = cv[:, i0:i0 + 2, dh:dh + 16, dw:dw + 16]
                    nc.tensor.matmul(psm[:], w2c2[:, t * 128:(t + 1) * 128], rhs,
                                     start=(t == 0), stop=(t == 8))
                nc.vector.bn_stats(s_b2c2[:, cc * 6:(cc + 1) * 6], psm[:])
                nc.scalar.copy(y22v[:, i0:i0 + 2, :, :],
                               psm[:].rearrange("p (i h w) -> p i h w", i=2, h=16))
            [(a22, b22)] = bn_sync([s_b2c2[:]], [5], False)
            nc.vector.tensor_scalar_add(sk2[:], sk2[:], b22[:])
            nc.vector.scalar_tensor_tensor(y22[:], y22[:], a22[:], sk2[:],
                                           op0=OP.mult, op1=OP.add)
            b3pad = mtile((128, BL * 100), "padA", F32R)
            nc.sync.dma_start(b3pad[:], ZPAD[:, 0:BL * 100])
            b3padv = b3pad[:].rearrange("p (i h w) -> p i h w", i=BL, h=10)
            zp2 = mtile((128, BL * 64), "sE")
            zp2v = zp2[:].rearrange("p (i h w) -> p i h w", i=BL, h=8)
            pool2x2_all(y22v, zp2v, BL, 16, "sD")
            nc.scalar.activation(b3padv[:, :, 1:9, 1:9], zp2v, ACTF.Relu)

            # ================= B3 =================
            w3c1 = load(W3C1, (128, 18 * 128), "wconv")
            w3skip = load(W3SKIP, (128, 2 * 128), "wskip")
            s_b3c1 = stile((128, 24), "s_b3c1")
            s_b3sk = stile((128, 24), "s_b3sk")
            y31 = mtile((128, 2 * BL * 64), "sA")
            sk3 = mtile((128, 2 * BL * 64), "sB")
            y31v = y31[:].rearrange("p (m i h w) -> p m i h w", m=2, i=BL, h=8)
            sk3v = sk3[:].rearrange("p (m i h w) -> p m i h w", m=2, i=BL, h=8)
            for mh in range(2):
                for cc in range(2):
                    i0 = cc * 8
                    psm = pp.tile([128, 512], F32, tag="psm",
                                  name=f"psb3c1_{mh}_{cc}")
                    for t in range(9):
                        dh, dw = divmod(t, 3)
                        rhs = b3padv[:, i0:i0 + 8, dh:dh + 8, dw:dw + 8]
                        nc.tensor.matmul(
                            psm[:],
                            w3c1[:, (t * 2 + mh) * 128:(t * 2 + mh + 1) * 128],
                            rhs, start=(t == 0), stop=(t == 8))
                    ci = mh * 2 + cc
                    nc.vector.bn_stats(s_b3c1[:, ci * 6:(ci + 1) * 6], psm[:])
                    nc.scalar.copy(y31v[:, mh, i0:i0 + 8, :, :],
                                   psm[:].rearrange("p (i h w) -> p i h w",
                                                    i=8, h=8))
                    psk = pps.tile([128, 512], F32, tag="psk",
                                  name=f"psb3sk_{mh}_{cc}")
                    rhs = b3padv[:, i0:i0 + 8, 1:9, 1:9]
                    nc.tensor.matmul(psk[:], w3skip[:, mh * 128:(mh + 1) * 128],
                                     rhs, start=True, stop=True)
                    nc.vector.bn_stats(s_b3sk[:, ci * 6:(ci + 1) * 6], psk[:])
                    nc.scalar.copy(sk3v[:, mh, i0:i0 + 8, :, :],
                                   psk[:].rearrange("p (i h w) -> p i h w",
                                                    i=8, h=8))
            sc1 = s_b3c1[:].rearrange("p (m f) -> p m f", m=2)
            ssk = s_b3sk[:].rearrange("p (m f) -> p m f", m=2)
            abs3 = bn_sync([sc1[:, 0, :], sc1[:, 1, :], ssk[:, 0, :],
                            ssk[:, 1, :]], [6, 7, 8, 9], False)
            b3c1pad = mtile((128, 2 * BL * 100), "padB", F32R)
            nc.sync.dma_start(b3c1pad[:], ZPAD[:, 0:2 * BL * 100])
            c1v = b3c1pad[:].rearrange("p (m i h w) -> p m i h w", m=2, i=BL,
                                       h=10)
            sk3m = sk3[:].rearrange("p (m f) -> p m f", m=2)
            for mh in range(2):
                nc.scalar.activation(c1v[:, mh, :, 1:9, 1:9], y31v[:, mh],
                                     ACTF.Relu, bias=abs3[mh][1][:],
                                     scale=abs3[mh][0][:])
                nc.vector.tensor_scalar(sk3m[:, mh, :], sk3m[:, mh, :],
                                        abs3[2 + mh][0][:], abs3[2 + mh][1][:],
                                        op0=OP.mult, op1=OP.add)

            w3c2 = load(W3C2, (128, 36 * 128), "wconv")
            s_b3c2 = stile((128, 24), "s_b3c2")
            y32 = mtile((128, 2 * BL * 64), "sC")
            y32v = y32[:].rearrange("p (m i h w) -> p m i h w", m=2, i=BL, h=8)
            for mh in range(2):
                for cc in range(2):
                    i0 = cc * 8
                    psm = pp.tile([128, 512], F32, tag="psm",
                                  name=f"psb3c2_{mh}_{cc}")
                    first = True
                    for t in range(9):
                        dh, dw = divmod(t, 3)
                        for kc in range(2):
                            rhs = c1v[:, kc, i0:i0 + 8, dh:dh + 8, dw:dw + 8]
                            idx = t * 4 + kc * 2 + mh
                            nc.tensor.matmul(psm[:],
                                             w3c2[:, idx * 128:(idx + 1) * 128],
                                             rhs, start=first,
                                             stop=(t == 8 and kc == 1))
                            first = False
                    ci = mh * 2 + cc
                    nc.vector.bn_stats(s_b3c2[:, ci * 6:(ci + 1) * 6], psm[:])
                    nc.scalar.copy(y32v[:, mh, i0:i0 + 8, :, :],
                                   psm[:].rearrange("p (i h w) -> p i h w",
                                                    i=8, h=8))
            sc2 = s_b3c2[:].rearrange("p (m f) -> p m f", m=2)
            abs32 = bn_sync([sc2[:, 0, :], sc2[:, 1, :]], [10, 11], False)
            y32m = y32[:].rearrange("p (m f) -> p m f", m=2)
            for mh in range(2):
                nc.vector.tensor_scalar_add(sk3m[:, mh, :], sk3m[:, mh, :],
                                            abs32[mh][1][:])
                nc.vector.scalar_tensor_tensor(y32m[:, mh, :], y32m[:, mh, :],
                                               abs32[mh][0][:], sk3m[:, mh, :],
                                               op0=OP.mult, op1=OP.add)
            nc.vector.tensor_scalar_max(y32[:], y32[:], 0.0)
            featf = stile((128, 2 * BL), "featf")
            nc.vector.tensor_reduce(
                featf[:], y32[:].rearrange("p (mi s) -> p mi s", s=64),
                axis=AX.X, op=OP.add)
            featr = mem.tile([128, 2 * BL], F32R, tag="featr", name="featr")
            nc.vector.tensor_copy(featr[:], featf[:])

            # ================= HEADS =================
            wchar = load(WCHAR, (128, 2 * 3840), "sB")
            wsmall = load(WSMALL, (128, 2 * 312), "sD")
            bchar = load(BCHAR, (128, 30), "bchar", F32)
            bsm = load(BSM, (128, 4), "bsm", F32)
            bstr_t = load(BSTR, (128, 1), "bstr", F32)

            def head_mm(wtile, base, mpad, mc, btile, bcol, out_dram, olim):
                mv_ = min(128, mpad - mc * 128)
                nv = min(mv_, olim - mc * 128)
                psm = pps.tile([128, BL], F32, tag="psk",
                              name=f"headps_{tctr[0]}_{mc}")
                for kc in range(2):
                    nc.tensor.matmul(
                        psm[0:mv_, :],
                        wtile[:, base + kc * mpad + mc * 128:
                              base + kc * mpad + mc * 128 + mv_],
                        featr[:, kc * BL:(kc + 1) * BL],
                        start=(kc == 0), stop=(kc == 1))
                tctr[0] += 1
                sb = hpool.tile([128, BL], F32, tag="headsb",
                             name=f"headsb_{tctr[0]}")
                nc.vector.tensor_scalar_add(sb[0:nv, :], psm[0:nv, :],
                                            btile[0:nv, bcol:bcol + 1])
                nc.sync.dma_start(
                    out_dram.rearrange("i o -> o i")[mc * 128: mc * 128 + nv, :],
                    sb[0:nv, :])

            for mc in range(30):
                head_mm(wchar, 0, 3840, mc, bchar, mc, CHAR, NUM_CLASSES)
            # WSMALL column layout: rad: [0, 512) as 2x256; sc: [512, 576) 2x32;
            # st: [576, 592) 2x8; str: [592, 624) 2x16
            for mc in range(2):
                head_mm(wsmall, 0, 256, mc, bsm, mc, RAD, NUM_RADICALS)
            head_mm(wsmall, 512, 32, 0, bsm, 2, SCO, NUM_SC)
            head_mm(wsmall, 576, 8, 0, bsm, 3, STO, NUM_ST)
            head_mm(wsmall, 592, 16, 0, bstr_t, 0, STRO, NUM_STRUCT)

    nc.compile()
    return nc


# ---------------------------------------------------------------- host tail

def _rerank_host(char, rad, sc, st, strl, params, radical_mask,
                 structure_label, stroke_count_label):
    p = params
    dt = np.float32
    idx = np.argsort(-char, axis=1, kind="stable")[:, :TOP_K]
    preds = (rad > 0).astype(dt)          # sigmoid(x) > 0.5  <=>  x > 0
    mask = radical_mask[idx].astype(dt)
    detected = np.einsum("br,bkr->bk", preds, mask)
    cnt = np.clip(mask.sum(-1), 1.0, None)
    match_ratio = detected / cnt
    num_det = preds.sum(-1, keepdims=True)
    false_alarms = np.einsum("br,bkr->bk", preds, 1.0 - mask)
    false_ratio = false_alarms / np.clip(num_det, 1.0, None)
    e = np.exp(strl - strl.max(-1, keepdims=True))
    sp = e / e.sum(-1, keepdims=True)
    structure_match = np.take_along_axis(sp, structure_label[idx], 1)
    stroke_pred = np.argmax(sc, -1)
    stroke_dist = np.abs(stroke_pred[:, None] - stroke_count_label[idx]).astype(dt) / 29.0

    def l2n(x):
        n = np.sqrt((x * x).sum(-1, keepdims=True))
        return x / np.clip(n, 1e-12, None)

    pn = l2n(st)
    sn = l2n(np.asarray(p["stroke_type_sig"], dt))[idx]
    cos = np.einsum("bd,bkd->bk", pn, sn)
    feats = np.stack([match_ratio, false_ratio, structure_match,
                      stroke_dist, cos], -1).astype(dt)
    h = np.maximum(feats @ np.asarray(p["mlp_w1"], dt).T +
                   np.asarray(p["mlp_b1"], dt), 0)
    scores = (h @ np.asarray(p["mlp_w2"], dt).T + np.asarray(p["mlp_b2"], dt))[..., 0]
    top_logits = np.take_along_axis(char, idx, 1)
    combined = top_logits + np.float32(p["rr_w"]) * scores
    out = char.copy()
    np.put_along_axis(out, idx, combined, axis=1)
    return out


# ---------------------------------------------------------------- entry point

def kernel(x, params, radical_mask, structure_label, stroke_count_label,
           _n_cores=N_CORES):
    x = np.asarray(x, np.float32)
    radical_mask = np.asarray(radical_mask)
    structure_label = np.asarray(structure_label)
    stroke_count_label = np.asarray(stroke_count_label)

    key = ("nc", _n_cores)
    if key not in _CACHE:
        _CACHE[key] = _build(_n_cores)
    nc = _CACHE[key]

    consts = _prep_consts(params)
    bsz = x.shape[0] // _n_cores
    in_maps = []
    for c in range(_n_cores):
        m = dict(consts)
        m["X0"] = _prep_stem_rhs(x[c * bsz:(c + 1) * bsz])
        in_maps.append(m)

    res = run_bass_kernel_spmd(nc, in_maps, core_ids=list(range(_n_cores)))
    cat = lambda k: np.concatenate([res.results[c][k] for c in range(_n_cores)], 0)
    return _rerank_host(cat("CHAR"), cat("RAD"), cat("SCO"), cat("STO"),
                        cat("STRO"), params, radical_mask, structure_label,
                        stroke_count_label)


# revision 14
# speedup vs baseline: 1.0375x; 1.0375x over previous
"""Trainium2 Bass kernel for nn_MiniResNetJoint (topk_masking).

Data-parallel over 8 NeuronCores (16 images each). BN batch statistics are
synchronized across cores with small AllGather collectives (7 rounds).
Conv trunk + 5 linear heads run on device with fp32r (s1e8m11) matmuls and
fp32 accumulation; the tiny symbolic reranker tail runs on host numpy.
"""

import numpy as np

import concourse.bass as bass
import concourse.mybir as mybir
import concourse.tile as tile
from concourse import bacc
from concourse.bass_utils import run_bass_kernel_spmd

F32 = mybir.dt.float32
F32R = mybir.dt.float32r
AX = mybir.AxisListType
OP = mybir.AluOpType
ACTF = mybir.ActivationFunctionType

N_CORES = 8
BL = 16            # local images per core
HALF = 8           # images per partition-half for 64ch layers
NUM_CLASSES = 3755
NUM_RADICALS = 214
NUM_STRUCT = 13
NUM_SC = 30
NUM_ST = 6
TOP_K = 5
EPS = 1e-5

_CACHE = {}


def _round_f32r(x):
    xi = np.ascontiguousarray(x, dtype=np.float32).view(np.uint32).astype(np.uint64)
    shift = np.uint64(12)
    add = np.uint64((1 << 11) - 1)
    lsb = (xi >> shift) & np.uint64(1)
    xi = (((xi + add + lsb) >> shift) << shift).astype(np.uint32)
    return xi.view(np.float32).reshape(np.shape(x))


# ---------------------------------------------------------------- host prep

def _prep_consts(params):
    p = {k: np.asarray(v) for k, v in params.items()}
    c = {}

    ws = np.asarray(p["stem_w"], np.float32)[:, 0]          # (64,3,3)
    lst = np.zeros((18, 128), np.float32)
    for t in range(9):
        dh, dw = divmod(t, 3)
        lst[t, :64] = ws[:, dh, dw]
        lst[9 + t, 64:] = ws[:, dh, dw]
    c["WSTEM"] = _round_f32r(lst)

    def blockdiag(w_tap):                                   # (64,64)->(128,128)
        out = np.zeros((128, 128), np.float32)
        out[:64, :64] = w_tap.T
        out[64:, 64:] = w_tap.T
        return out

    def duprows(w_tap):                                     # (128,64)->(128,128)
        out = np.zeros((128, 128), np.float32)
        out[:64, :] = w_tap.T
        out[64:, :] = w_tap.T
        return out

    for name, key in (("W1C1", "b1_c1"), ("W1C2", "b1_c2")):
        w = np.asarray(p[key], np.float32)
        taps = np.stack([blockdiag(w[:, :, t // 3, t % 3]) for t in range(9)])
        c[name] = _round_f32r(taps.transpose(1, 0, 2).reshape(128, 9 * 128))

    w = np.asarray(p["b2_c1"], np.float32)
    taps = np.stack([duprows(w[:, :, t // 3, t % 3]) for t in range(9)])
    c["W2C1"] = _round_f32r(taps.transpose(1, 0, 2).reshape(128, 9 * 128))
    c["W2SKIP"] = _round_f32r(duprows(np.asarray(p["b2_skip_w"], np.float32)[:, :, 0, 0]))

    w = np.asarray(p["b2_c2"], np.float32)
    taps = np.stack([w[:, :, t // 3, t % 3].T for t in range(9)])
    c["W2C2"] = _round_f32r(taps.transpose(1, 0, 2).reshape(128, 9 * 128))

    w = np.asarray(p["b3_c1"], np.float32)
    arr = np.zeros((9, 2, 128, 128), np.float32)
    for t in range(9):
        for mh in range(2):
            arr[t, mh] = w[mh * 128:(mh + 1) * 128, :, t // 3, t % 3].T
    c["W3C1"] = _round_f32r(arr.reshape(18, 128, 128).transpose(1, 0, 2)
                            .reshape(128, 18 * 128))

    w = np.asarray(p["b3_skip_w"], np.float32)[:, :, 0, 0]
    arr = np.stack([w[:128].T, w[128:].T])
    c["W3SKIP"] = _round_f32r(arr.transpose(1, 0, 2).reshape(128, 2 * 128))

    w = np.asarray(p["b3_c2"], np.float32)
    arr = np.zeros((9, 2, 2, 128, 128), np.float32)
    for t in range(9):
        for kc in range(2):
            for mh in range(2):
                arr[t, kc, mh] = w[mh * 128:(mh + 1) * 128,
                                   kc * 128:(kc + 1) * 128, t // 3, t % 3].T
    c["W3C2"] = _round_f32r(arr.reshape(36, 128, 128).transpose(1, 0, 2)
                            .reshape(128, 36 * 128))

    def head(wk_, mpad):
        w_ = np.asarray(p[wk_], np.float32) / 64.0          # GAP fold
        o = w_.shape[0]
        arr_ = np.zeros((2, 128, mpad), np.float32)
        for kc in range(2):
            arr_[kc, :, :o] = w_[:, kc * 128:(kc + 1) * 128].T
        return _round_f32r(arr_.transpose(1, 0, 2).reshape(128, 2 * mpad))

    c["WCHAR"] = head("char_w", 3840)
    c["WSMALL"] = np.concatenate([head("rad_w", 256), head("sc_w", 32),
                                  head("st_w", 8), head("str_w", 16)], axis=1)

    bc = np.zeros((30, 128), np.float32)
    bc.reshape(-1)[:NUM_CLASSES] = np.asarray(p["char_b"], np.float32)
    c["BCHAR"] = np.ascontiguousarray(bc.T)                  # [128, 30]
    bsm = np.zeros((128, 4), np.float32)
    rb = np.asarray(p["rad_b"], np.float32)
    bsm[:128, 0] = rb[:128]
    bsm[:86, 1] = rb[128:]
    bsm[:30, 2] = np.asarray(p["sc_b"], np.float32)
    bsm[:6, 3] = np.asarray(p["st_b"], np.float32)
    c["BSM"] = bsm
    bstr = np.zeros((128, 1), np.float32)
    bstr[:13, 0] = np.asarray(p["str_b"], np.float32)
    c["BSTR"] = bstr

    def gb_dup(g, b_):
        g = np.asarray(g, np.float32); b_ = np.asarray(b_, np.float32)
        return np.stack([np.concatenate([g, g]), np.concatenate([b_, b_])], 1)

    def gb_full(g, b_):
        return np.stack([np.asarray(g, np.float32), np.asarray(b_, np.float32)], 1)

    c["ZPAD"] = np.zeros((128, 34 * 34 * 8), np.float32)
    c["GBALL"] = np.concatenate([
        gb_dup(p["stem_g"], p["stem_b"]),
        gb_dup(p["b1_g1"], p["b1_beta1"]),
        gb_dup(p["b1_g2"], p["b1_beta2"]),
        gb_full(p["b2_g1"], p["b2_beta1"]),
        gb_full(p["b2_skip_g"], p["b2_skip_b"]),
        gb_full(p["b2_g2"], p["b2_beta2"]),
        gb_full(p["b3_g1"][:128], p["b3_beta1"][:128]),
        gb_full(p["b3_g1"][128:], p["b3_beta1"][128:]),
        gb_full(p["b3_skip_g"][:128], p["b3_skip_b"][:128]),
        gb_full(p["b3_skip_g"][128:], p["b3_skip_b"][128:]),
        gb_full(p["b3_g2"][:128], p["b3_beta2"][:128]),
        gb_full(p["b3_g2"][128:], p["b3_beta2"][128:]),
    ], axis=1).astype(np.float32)                           # [128, 24]
    return c


def _prep_stem_rhs(x_core):
    """(16,1,64,64) -> [8(pass),18,4356] f32r rolled zero-padded planes."""
    x = np.asarray(x_core, np.float32)[:, 0]
    xp = np.zeros((16, 66, 66), np.float32)
    xp[:, 1:65, 1:65] = x
    flat = xp.reshape(16, 66 * 66)
    out = np.zeros((8, 18, 4356), np.float32)
    for ps in range(8):
        for half in range(2):
            img = half * 8 + ps
            for t in range(9):
                dh, dw = divmod(t, 3)
                s = dh * 66 + dw
                seg = flat[img, s:s + 4356]
                out[ps, half * 9 + t, :len(seg)] = seg
    return _round_f32r(out)


# ---------------------------------------------------------------- device build

def _build(n_cores):
    nc = bacc.Bacc("TRN2", target_bir_lowering=False, debug=False,
                   num_devices=n_cores)

    def din(name, shape, dtype=F32R):
        return nc.dram_tensor(name, list(shape), dtype, kind="ExternalInput").ap()

    X0 = din("X0", (8, 18, 4356))
    WSTEM = din("WSTEM", (18, 128))
    W1C1 = din("W1C1", (128, 9 * 128))
    W1C2 = din("W1C2", (128, 9 * 128))
    W2C1 = din("W2C1", (128, 9 * 128))
    W2SKIP = din("W2SKIP", (128, 128))
    W2C2 = din("W2C2", (128, 9 * 128))
    W3C1 = din("W3C1", (128, 18 * 128))
    W3SKIP = din("W3SKIP", (128, 2 * 128))
    W3C2 = din("W3C2", (128, 36 * 128))
    WCHAR = din("WCHAR", (128, 2 * 3840))
    WSMALL = din("WSMALL", (128, 2 * 312))
    BCHAR = din("BCHAR", (128, 30), F32)
    BSM = din("BSM", (128, 4), F32)
    BSTR = din("BSTR", (128, 1), F32)
    GBALL = din("GBALL", (128, 24), F32)
    ZPAD = din("ZPAD", (128, 34 * 34 * 8))

    CHAR = nc.dram_tensor("CHAR", [BL, NUM_CLASSES], F32, kind="ExternalOutput").ap()
    RAD = nc.dram_tensor("RAD", [BL, NUM_RADICALS], F32, kind="ExternalOutput").ap()
    SCO = nc.dram_tensor("SCO", [BL, NUM_SC], F32, kind="ExternalOutput").ap()
    STO = nc.dram_tensor("STO", [BL, NUM_ST], F32, kind="ExternalOutput").ap()
    STRO = nc.dram_tensor("STRO", [BL, NUM_STRUCT], F32, kind="ExternalOutput").ap()

    rg = [list(range(n_cores))]

    with tile.TileContext(nc) as tc:
        with (
            tc.tile_pool(name="mem", bufs=1) as mem,
            tc.tile_pool(name="psum", bufs=6, space="PSUM") as pp,
            tc.tile_pool(name="psum2", bufs=2, space="PSUM") as pps,
            tc.tile_pool(name="dram", bufs=2, space="DRAM") as dpool,
            tc.tile_pool(name="hout", bufs=4) as hpool,
            tc.tile_pool(name="stats", bufs=1) as spool,
        ):
            tctr = [0]

            def mtile(shape, slot, dtype=F32):
                tctr[0] += 1
                return mem.tile(list(shape), dtype, tag=slot,
                                name=f"{slot}_{tctr[0]}")

            def load(ap_dram, shape, slot, dtype=F32R):
                t = mtile(shape, slot, dtype)
                nc.sync.dma_start(t[:], ap_dram)
                return t

            # persistent small constants
            gball = load(GBALL, (128, 24), "gball", F32)

            uid = [0]

            def stile(shape, name):
                return spool.tile(list(shape), F32, tag=name, name=name)

            def rsqrt_newton(out_ap, var_ap):
                uid[0] += 1
                u = uid[0]
                t = stile((128, 1), f"rsq_t{u}")
                s = stile((128, 1), f"rsq_s{u}")
                r0 = stile((128, 1), f"rsq_r0{u}")
                h = stile((128, 1), f"rsq_h{u}")
                nc.vector.tensor_scalar_add(t[:], var_ap, float(EPS))
                nc.scalar.activation(s[:], t[:], ACTF.Sqrt)
                nc.vector.reciprocal(r0[:], s[:])
                nc.vector.tensor_tensor(h[:], r0[:], r0[:], op=OP.mult)
                nc.vector.tensor_tensor(h[:], h[:], t[:], op=OP.mult)
                nc.vector.tensor_scalar(h[:], h[:], -0.5, 1.5, op0=OP.mult,
                                        op1=OP.add)
                nc.vector.tensor_tensor(out_ap, r0[:], h[:], op=OP.mult)

            def bn_sync(stat_groups, gb_cols, dup_halves):
                """Returns [(a_tile, b_tile)] per group; a/b are [128,1] fp32."""
                uid[0] += 1
                u = uid[0]
                L = len(stat_groups)
                mv = stile((128, 2 * L), f"mv{u}")
                for gi, sg in enumerate(stat_groups):
                    ag = stile((128, 2), f"aggr{u}_{gi}")
                    nc.vector.bn_aggr(ag[:], sg)
                    nc.vector.tensor_copy(mv[:, 2 * gi:2 * gi + 1], ag[:, 0:1])
                    m2 = stile((128, 1), f"m2_{u}_{gi}")
                    nc.vector.tensor_tensor(m2[:], ag[:, 0:1], ag[:, 0:1],
                                            op=OP.mult)
                    nc.vector.tensor_tensor(mv[:, 2 * gi + 1:2 * gi + 2],
                                            ag[:, 1:2], m2[:], op=OP.add)
                agin = dpool.tile([128, 2 * L], F32, tag="agin", name=f"agin{u}")
                agout = dpool.tile([n_cores * 128, 2 * L], F32,
                                   addr_space="Shared", tag="agout",
                                   name=f"agout{u}")
                nc.sync.dma_start(agin[:], mv[:])
                if n_cores > 1:
                    nc.gpsimd.collective_compute(
                        "AllGather", OP.bypass, replica_groups=rg,
                        ins=[agin[:].opt()], outs=[agout[:].opt()])
                else:
                    nc.sync.dma_start(agout[:], agin[:])
                gt = stile((128, 2 * L * n_cores), f"gt{u}")
                nc.sync.dma_start(
                    gt[:], agout[:].rearrange("(r p) j -> p j r", r=n_cores))
                red = stile((128, 2 * L), f"red{u}")
                nc.vector.tensor_reduce(
                    red[:], gt[:].rearrange("p (j r) -> p j r", r=n_cores),
                    axis=AX.X, op=OP.add)
                if dup_halves:
                    up = stile((64, 2 * L), f"up{u}")
                    nc.sync.dma_start(up[:], red[64:128, :])
                    nc.vector.tensor_tensor(red[0:64, :], red[0:64, :], up[:],
                                            op=OP.add)
                    nc.vector.tensor_scalar_mul(red[0:64, :], red[0:64, :],
                                                1.0 / (2 * n_cores))
                    nc.sync.dma_start(red[64:128, :], red[0:64, :])
                else:
                    nc.vector.tensor_scalar_mul(red[:], red[:], 1.0 / n_cores)
                out = []
                for gi in range(L):
                    m = red[:, 2 * gi:2 * gi + 1]
                    ex2 = red[:, 2 * gi + 1:2 * gi + 2]
                    var = stile((128, 1), f"var{u}_{gi}")
                    nc.vector.tensor_tensor(var[:], m, m, op=OP.mult)
                    nc.vector.tensor_tensor(var[:], ex2, var[:], op=OP.subtract)
                    r = stile((128, 1), f"r{u}_{gi}")
                    rsqrt_newton(r[:], var[:])
                    a = stile((128, 1), f"a{u}_{gi}")
                    b = stile((128, 1), f"b{u}_{gi}")
                    gcol = gb_cols[gi]
                    nc.vector.tensor_tensor(a[:], gball[:, 2 * gcol:2 * gcol + 1],
                                            r[:], op=OP.mult)
                    nc.vector.tensor_tensor(b[:], m, a[:], op=OP.mult)
                    nc.vector.tensor_tensor(b[:],
                                            gball[:, 2 * gcol + 1:2 * gcol + 2],
                                            b[:], op=OP.subtract)
                    out.append((a, b))
                return out

            def pool2x2_all(src4, dst4, n_img, hw, slot):
                h2 = hw // 2
                tmp = mtile((128, n_img * hw * h2), slot)
                tv = tmp[:].rearrange("p (i h w) -> p i h w", i=n_img, h=hw)
                s = src4.rearrange("p i h (w2 two) -> p i h w2 two", two=2)
                nc.vector.tensor_tensor(tv, s[:, :, :, :, 0], s[:, :, :, :, 1],
                                        op=OP.max)
                t2 = tv.rearrange("p i (h2 two) w -> p i h2 two w", two=2)
                nc.vector.tensor_tensor(dst4, t2[:, :, :, 0, :],
                                        t2[:, :, :, 1, :], op=OP.max)

            # ================= STEM =================
            wstem = load(WSTEM, (18, 128), "wstem")
            b1pad = mtile((128, HALF * 34 * 34), "padA", F32R)
            nc.sync.dma_start(b1pad[:], ZPAD[:, 0:HALF * 34 * 34])
            b1padv = b1pad[:].rearrange("p (i h w) -> p i h w", i=HALF, h=34)
            st_stats = stile((128, 64 * 6), "st_stats")

            for ps in range(8):
                x0t = load(X0[ps], (18, 4356), "sA")
                raw = mtile((128, 4096), "sB")
                rawv = raw[:].rearrange("p (h w) -> p h w", h=64)
                for hb in range(8):
                    psm = pp.tile([128, 512], F32, tag="psm", name=f"ps_st{ps}_{hb}")
                    rhs = x0t[:, hb * 528:(hb + 1) * 528].rearrange(
                        "p (h w) -> p h w", h=8)[:, :, 0:64]
                    nc.tensor.matmul(psm[:], wstem[:], rhs, start=True, stop=True)
                    ci = ps * 8 + hb
                    nc.vector.bn_stats(st_stats[:, ci * 6:(ci + 1) * 6], psm[:])
                    nc.scalar.copy(rawv[:, hb * 8:(hb + 1) * 8, :],
                                   psm[:].rearrange("p (h w) -> p h w", h=8))
                dst = b1padv[:, ps:ps + 1, 1:33, 1:33]
                pool2x2_all(rawv.rearrange("p h w -> p 1 h w" if False else
                                           "p h w -> p h w").unsqueeze(1)
                            if False else
                            raw[:].rearrange("p (i h w) -> p i h w", i=1, h=64),
                            dst, 1, 64, "sC")

            [(a0, b0)] = bn_sync([st_stats[:]], [0], True)
            intr = b1padv[:, :, 1:33, 1:33]
            nc.scalar.activation(intr, intr.bitcast(F32), ACTF.Relu,
                                 bias=b0[:], scale=a0[:])

            # ================= B1 =================
            def conv64(wtile, srcv, n_img, hw, dstv, stats_tile):
                rpc = 512 // hw
                nch = hw // rpc
                ci = 0
                for i in range(n_img):
                    for cch in range(nch):
                        psm = pp.tile([128, rpc * hw], F32, tag="psm",
                                      name=f"psc64_{tctr[0]}_{ci}")
                        for t in range(9):
                            dh, dw = divmod(t, 3)
                            rhs = srcv[:, i, dh + cch * rpc: dh + cch * rpc + rpc,
                                       dw:dw + hw]
                            nc.tensor.matmul(psm[:],
                                             wtile[:, t * 128:(t + 1) * 128],
                                             rhs, start=(t == 0), stop=(t == 8))
                        nc.vector.bn_stats(stats_tile[:, ci * 6:(ci + 1) * 6],
                                           psm[:])
                        nc.scalar.copy(
                            dstv[:, i, cch * rpc:(cch + 1) * rpc, :],
                            psm[:].rearrange("p (h w) -> p h w", h=rpc))
                        ci += 1

            w1c1 = load(W1C1, (128, 9 * 128), "wconvA")
            s_b1c1 = stile((128, 16 * 6), "s_b1c1")
            y1 = mtile((128, HALF * 1024), "sA")
            y1v = y1[:].rearrange("p (i h w) -> p i h w", i=HALF, h=32)
            conv64(w1c1, b1padv, HALF, 32, y1v, s_b1c1)
            [(a1, b1)] = bn_sync([s_b1c1[:]], [1], True)
            y1pad = mtile((128, HALF * 34 * 34), "padB", F32R)
            nc.sync.dma_start(y1pad[:], ZPAD[:, 0:HALF * 34 * 34])
            y1padv = y1pad[:].rearrange("p (i h w) -> p i h w", i=HALF, h=34)
            nc.scalar.activation(y1padv[:, :, 1:33, 1:33], y1v, ACTF.Relu,
                                 bias=b1[:], scale=a1[:])

            w1c2 = load(W1C2, (128, 9 * 128), "wconvB")
            s_b1c2 = stile((128, 16 * 6), "s_b1c2")
            y2 = mtile((128, HALF * 1024), "sB")
            y2v = y2[:].rearrange("p (i h w) -> p i h w", i=HALF, h=32)
            conv64(w1c2, y1padv, HALF, 32, y2v, s_b1c2)
            [(a2, b2)] = bn_sync([s_b1c2[:]], [2], True)
            zb1 = mtile((128, HALF * 1024), "sA")
            zb1v = zb1[:].rearrange("p (i h w) -> p i h w", i=HALF, h=32)
            nc.vector.tensor_scalar_add(zb1v, intr.bitcast(F32), b2[:])
            nc.vector.scalar_tensor_tensor(zb1[:], y2[:], a2[:], zb1[:],
                                           op0=OP.mult, op1=OP.add)
            b2pad = mtile((128, HALF * 18 * 18), "padA", F32R)
            nc.sync.dma_start(b2pad[:], ZPAD[:, 0:HALF * 18 * 18])
            b2padv = b2pad[:].rearrange("p (i h w) -> p i h w", i=HALF, h=18)
            zp1 = mtile((128, HALF * 256), "sE")
            zp1v = zp1[:].rearrange("p (i h w) -> p i h w", i=HALF, h=16)
            pool2x2_all(zb1v, zp1v, HALF, 32, "sC")
            nc.scalar.activation(b2padv[:, :, 1:17, 1:17], zp1v, ACTF.Relu)

            # ================= B2 =================
            w2c1 = load(W2C1, (128, 9 * 128), "wconvA")
            w2skip = load(W2SKIP, (128, 128), "wskip")
            s_b2c1 = stile((128, 8 * 6), "s_b2c1")
            s_b2sk = stile((128, 8 * 6), "s_b2sk")
            y21 = mtile((128, BL * 256), "sA")
            sk2 = mtile((128, BL * 256), "sB")
            y21v = y21[:].rearrange("p (i h w) -> p i h w", i=BL, h=16)
            sk2v = sk2[:].rearrange("p (i h w) -> p i h w", i=BL, h=16)
            ci1 = 0
            for half in range(2):
                hp = half * 64
                for cc in range(4):
                    i0 = cc * 2
                    img = half * 8 + i0
                    psm = pp.tile([128, 512], F32, tag="psm", name=f"psb2c1_{ci1}")
                    for t in range(9):
                        dh, dw = divmod(t, 3)
                        rhs = b2padv[hp:hp + 64, i0:i0 + 2, dh:dh + 16,
                                     dw:dw + 16]
                        nc.tensor.matmul(psm[:],
                                         w2c1[hp:hp + 64, t * 128:(t + 1) * 128],
                                         rhs, start=(t == 0), stop=(t == 8))
                    nc.vector.bn_stats(s_b2c1[:, ci1 * 6:(ci1 + 1) * 6], psm[:])
                    nc.scalar.copy(y21v[:, img:img + 2, :, :],
                                   psm[:].rearrange("p (i h w) -> p i h w",
                                                    i=2, h=16))
                    psk = pps.tile([128, 512], F32, tag="psk", name=f"psb2sk_{ci1}")
                    rhs = b2padv[hp:hp + 64, i0:i0 + 2, 1:17, 1:17]
                    nc.tensor.matmul(psk[:], w2skip[hp:hp + 64, :], rhs,
                                     start=True, stop=True)
                    nc.vector.bn_stats(s_b2sk[:, ci1 * 6:(ci1 + 1) * 6], psk[:])
                    nc.scalar.copy(sk2v[:, img:img + 2, :, :],
                                   psk[:].rearrange("p (i h w) -> p i h w",
                                                    i=2, h=16))
                    ci1 += 1
            ab21, absk = bn_sync([s_b2c1[:], s_b2sk[:]], [3, 4], False)
            b2c1pad = mtile((128, BL * 18 * 18), "padB", F32R)
            nc.sync.dma_start(b2c1pad[:], ZPAD[:, 0:BL * 18 * 18])
            cv = b2c1pad[:].rearrange("p (i h w) -> p i h w", i=BL, h=18)
            nc.scalar.activation(cv[:, :, 1:17, 1:17], y21v, ACTF.Relu,
                                 bias=ab21[1][:], scale=ab21[0][:])
            nc.vector.tensor_scalar(sk2[:], sk2[:], absk[0][:], absk[1][:],
                                    op0=OP.mult, op1=OP.add)

            w2c2 = load(W2C2, (128, 9 * 128), "wconvB")
            s_b2c2 = stile((128, 8 * 6), "s_b2c2")
            y22 = mtile((128, BL * 256), "sC")
            y22v = y22[:].rearrange("p (i h w) -> p i h w", i=BL, h=16)
            for cc in range(8):
                i0 = cc * 2
                psm = pp.tile([128, 512], F32, tag="psm", name=f"psb2c2_{cc}")
                for t in range(9):
                    dh, dw = divmod(t, 3)
                    rhs = cv[:, i0:i0 + 2, dh:dh + 16, dw:dw + 16]
                    nc.tensor.matmul(psm[:], w2c2[:, t * 128:(t + 1) * 128], rhs,
                                     start=(t == 0), stop=(t == 8))
                nc.vector.bn_stats(s_b2c2[:, cc * 6:(cc + 1) * 6], psm[:])
                nc.scalar.copy(y22v[:, i0:i0 + 2, :, :],
                               psm[:].rearrange("p (i h w) -> p i h w", i=2, h=16))
            [(a22, b22)] = bn_sync([s_b2c2[:]], [5], False)
            nc.vector.tensor_scalar_add(sk2[:], sk2[:], b22[:])
            nc.vector.scalar_tensor_tensor(y22[:], y22[:], a22[:], sk2[:],
                                           op0=OP.mult, op1=OP.add)
            b3pad = mtile((128, BL * 100), "padA", F32R)
            nc.sync.dma_start(b3pad[:], ZPAD[:, 0:BL * 100])
            b3padv = b3pad[:].rearrange("p (i h w) -> p i h w", i=BL, h=10)
            zp2 = mtile((128, BL * 64), "sE")
            zp2v = zp2[:].rearrange("p (i h w) -> p i h w", i=BL, h=8)
            pool2x2_all(y22v, zp2v, BL, 16, "sD")
            nc.scalar.activation(b3padv[:, :, 1:9, 1:9], zp2v, ACTF.Relu)

            # ================= B3 =================
            w3c1 = load(W3C1, (128, 18 * 128), "wconvA")
            w3skip = load(W3SKIP, (128, 2 * 128), "wskip")
            s_b3c1 = stile((128, 24), "s_b3c1")
            s_b3sk = stile((128, 24), "s_b3sk")
            y31 = mtile((128, 2 * BL * 64), "sA")
            sk3 = mtile((128, 2 * BL * 64), "sB")
            y31v = y31[:].rearrange("p (m i h w) -> p m i h w", m=2, i=BL, h=8)
            sk3v = sk3[:].rearrange("p (m i h w) -> p m i h w", m=2, i=BL, h=8)
            for mh in range(2):
                for cc in range(2):
                    i0 = cc * 8
                    psm = pp.tile([128, 512], F32, tag="psm",
                                  name=f"psb3c1_{mh}_{cc}")
                    for t in range(9):
                        dh, dw = divmod(t, 3)
                        rhs = b3padv[:, i0:i0 + 8, dh:dh + 8, dw:dw + 8]
                        nc.tensor.matmul(
                            psm[:],
                            w3c1[:, (t * 2 + mh) * 128:(t * 2 + mh + 1) * 128],
                            rhs, start=(t == 0), stop=(t == 8))
                    ci = mh * 2 + cc
                    nc.vector.bn_stats(s_b3c1[:, ci * 6:(ci + 1) * 6], psm[:])
                    nc.scalar.copy(y31v[:, mh, i0:i0 + 8, :, :],
                                   psm[:].rearrange("p (i h w) -> p i h w",
                                                    i=8, h=8))
                    psk = pps.tile([128, 512], F32, tag="psk",
                                  name=f"psb3sk_{mh}_{cc}")
                    rhs = b3padv[:, i0:i0 + 8, 1:9, 1:9]
                    nc.tensor.matmul(psk[:], w3skip[:, mh * 128:(mh + 1) * 128],
                                     rhs, start=True, stop=True)
                    nc.vector.bn_stats(s_b3sk[:, ci * 6:(ci + 1) * 6], psk[:])
                    nc.scalar.copy(sk3v[:, mh, i0:i0 + 8, :, :],
                                   psk[:].rearrange("p (i h w) -> p i h w",
                                                    i=8, h=8))
            sc1 = s_b3c1[:].rearrange("p (m f) -> p m f", m=2)
            ssk = s_b3sk[:].rearrange("p (m f) -> p m f", m=2)
            abs3 = bn_sync([sc1[:, 0, :], sc1[:, 1, :], ssk[:, 0, :],
                            ssk[:, 1, :]], [6, 7, 8, 9], False)
            b3c1pad = mtile((128, 2 * BL * 100), "padB", F32R)
            nc.sync.dma_start(b3c1pad[:], ZPAD[:, 0:2 * BL * 100])
            c1v = b3c1pad[:].rearrange("p (m i h w) -> p m i h w", m=2, i=BL,
                                       h=10)
            sk3m = sk3[:].rearrange("p (m f) -> p m f", m=2)
            for mh in range(2):
                nc.scalar.activation(c1v[:, mh, :, 1:9, 1:9], y31v[:, mh],
                                     ACTF.Relu, bias=abs3[mh][1][:],
                                     scale=abs3[mh][0][:])
                nc.vector.tensor_scalar(sk3m[:, mh, :], sk3m[:, mh, :],
                                        abs3[2 + mh][0][:], abs3[2 + mh][1][:],
                                        op0=OP.mult, op1=OP.add)

            w3c2 = load(W3C2, (128, 36 * 128), "wconvB")
            s_b3c2 = stile((128, 24), "s_b3c2")
            y32 = mtile((128, 2 * BL * 64), "sC")
            y32v = y32[:].rearrange("p (m i h w) -> p m i h w", m=2, i=BL, h=8)
            for mh in range(2):
                for cc in range(2):
                    i0 = cc * 8
                    psm = pp.tile([128, 512], F32, tag="psm",
                                  name=f"psb3c2_{mh}_{cc}")
                    first = True
                    for t in range(9):
                        dh, dw = divmod(t, 3)
                        for kc in range(2):
                            rhs = c1v[:, kc, i0:i0 + 8, dh:dh + 8, dw:dw + 8]
                            idx = t * 4 + kc * 2 + mh
                            nc.tensor.matmul(psm[:],
                                             w3c2[:, idx * 128:(idx + 1) * 128],
                                             rhs, start=first,
                                             stop=(t == 8 and kc == 1))
                            first = False
                    ci = mh * 2 + cc
                    nc.vector.bn_stats(s_b3c2[:, ci * 6:(ci + 1) * 6], psm[:])
                    nc.scalar.copy(y32v[:, mh, i0:i0 + 8, :, :],
                                   psm[:].rearrange("p (i h w) -> p i h w",
                                                    i=8, h=8))
            sc2 = s_b3c2[:].rearrange("p (m f) -> p m f", m=2)
            abs32 = bn_sync([sc2[:, 0, :], sc2[:, 1, :]], [10, 11], False)
            y32m = y32[:].rearrange("p (m f) -> p m f", m=2)
            for mh in range(2):
                nc.vector.tensor_scalar_add(sk3m[:, mh, :], sk3m[:, mh, :],
                                            abs32[mh][1][:])
                nc.vector.scalar_tensor_tensor(y32m[:, mh, :], y32m[:, mh, :],
                                               abs32[mh][0][:], sk3m[:, mh, :],
                                               op0=OP.mult, op1=OP.add)
            nc.vector.tensor_scalar_max(y32[:], y32[:], 0.0)
            featf = stile((128, 2 * BL), "featf")
            nc.vector.tensor_reduce(
                featf[:], y32[:].rearrange("p (mi s) -> p mi s", s=64),
                axis=AX.X, op=OP.add)
            featr = mem.tile([128, 2 * BL], F32R, tag="featr", name="featr")
            nc.vector.tensor_copy(featr[:], featf[:])

            # ================= HEADS =================
            wchar = load(WCHAR, (128, 2 * 3840), "sB")
            wsmall = load(WSMALL, (128, 2 * 312), "sD")
            bchar = load(BCHAR, (128, 30), "bchar", F32)
            bsm = load(BSM, (128, 4), "bsm", F32)
            bstr_t = load(BSTR, (128, 1), "bstr", F32)

            def head_mm(wtile, base, mpad, mc, btile, bcol, out_dram, olim):
                mv_ = min(128, mpad - mc * 128)
                nv = min(mv_, olim - mc * 128)
                psm = pps.tile([128, BL], F32, tag="psk",
                              name=f"headps_{tctr[0]}_{mc}")
                for kc in range(2):
                    nc.tensor.matmul(
                        psm[0:mv_, :],
                        wtile[:, base + kc * mpad + mc * 128:
                              base + kc * mpad + mc * 128 + mv_],
                        featr[:, kc * BL:(kc + 1) * BL],
                        start=(kc == 0), stop=(kc == 1))
                tctr[0] += 1
                sb = hpool.tile([128, BL], F32, tag="headsb",
                             name=f"headsb_{tctr[0]}")
                nc.vector.tensor_scalar_add(sb[0:nv, :], psm[0:nv, :],
                                            btile[0:nv, bcol:bcol + 1])
                nc.sync.dma_start(
                    out_dram.rearrange("i o -> o i")[mc * 128: mc * 128 + nv, :],
                    sb[0:nv, :])

            for mc in range(30):
                head_mm(wchar, 0, 3840, mc, bchar, mc, CHAR, NUM_CLASSES)
            # WSMALL column layout: rad: [0, 512) as 2x256; sc: [512, 576) 2x32;
            # st: [576, 592) 2x8; str: [592, 624) 2x16
            for mc in range(2):
                head_mm(wsmall, 0, 256, mc, bsm, mc, RAD, NUM_RADICALS)
            head_mm(wsmall, 512, 32, 0, bsm, 2, SCO, NUM_SC)
            head_mm(wsmall, 576, 8, 0, bsm, 3, STO, NUM_ST)
            head_mm(wsmall, 592, 16, 0, bstr_t, 0, STRO, NUM_STRUCT)

    nc.compile()
    return nc


# ---------------------------------------------------------------- host tail

def _rerank_host(char, rad, sc, st, strl, params, radical_mask,
                 structure_label, stroke_count_label):
    p = params
    dt = np.float32
    idx = np.argsort(-char, axis=1, kind="stable")[:, :TOP_K]
    preds = (rad > 0).astype(dt)          # sigmoid(x) > 0.5  <=>  x > 0
    mask = radical_mask[idx].astype(dt)
    detected = np.einsum("br,bkr->bk", preds, mask)
    cnt = np.clip(mask.sum(-1), 1.0, None)
    match_ratio = detected / cnt
    num_det = preds.sum(-1, keepdims=True)
    false_alarms = np.einsum("br,bkr->bk", preds, 1.0 - mask)
    false_ratio = false_alarms / np.clip(num_det, 1.0, None)
    e = np.exp(strl - strl.max(-1, keepdims=True))
    sp = e / e.sum(-1, keepdims=True)
    structure_match = np.take_along_axis(sp, structure_label[idx], 1)
    stroke_pred = np.argmax(sc, -1)
    stroke_dist = np.abs(stroke_pred[:, None] - stroke_count_label[idx]).astype(dt) / 29.0

    def l2n(x):
        n = np.sqrt((x * x).sum(-1, keepdims=True))
        return x / np.clip(n, 1e-12, None)

    pn = l2n(st)
    sn = l2n(np.asarray(p["stroke_type_sig"], dt))[idx]
    cos = np.einsum("bd,bkd->bk", pn, sn)
    feats = np.stack([match_ratio, false_ratio, structure_match,
                      stroke_dist, cos], -1).astype(dt)
    h = np.maximum(feats @ np.asarray(p["mlp_w1"], dt).T +
                   np.asarray(p["mlp_b1"], dt), 0)
    scores = (h @ np.asarray(p["mlp_w2"], dt).T + np.asarray(p["mlp_b2"], dt))[..., 0]
    top_logits = np.take_along_axis(char, idx, 1)
    combined = top_logits + np.float32(p["rr_w"]) * scores
    out = char.copy()
    np.put_along_axis(out, idx, combined, axis=1)
    return out


# ---------------------------------------------------------------- entry point

def kernel(x, params, radical_mask, structure_label, stroke_count_label,
           _n_cores=N_CORES):
    x = np.asarray(x, np.float32)
    radical_mask = np.asarray(radical_mask)
    structure_label = np.asarray(structure_label)
    stroke_count_label = np.asarray(stroke_count_label)

    key = ("nc", _n_cores)
    if key not in _CACHE:
        _CACHE[key] = _build(_n_cores)
    nc = _CACHE[key]

    consts = _prep_consts(params)
    bsz = x.shape[0] // _n_cores
    in_maps = []
    for c in range(_n_cores):
        m = dict(consts)
        m["X0"] = _prep_stem_rhs(x[c * bsz:(c + 1) * bsz])
        in_maps.append(m)

    res = run_bass_kernel_spmd(nc, in_maps, core_ids=list(range(_n_cores)))
    cat = lambda k: np.concatenate([res.results[c][k] for c in range(_n_cores)], 0)
    return _rerank_host(cat("CHAR"), cat("RAD"), cat("SCO"), cat("STO"),
                        cat("STRO"), params, radical_mask, structure_label,
                        stroke_count_label)
